# revision 1
# baseline (speedup 1.0000x reference)
"""Trainium2 Bass kernel for a GQA attention block (B=2, S=2048, H=2048,
16 q-heads / 8 kv-heads, head_dim=128, fp32), tensor-parallel over heads
across 8 NeuronCores.

Per-core shard (core c): q-heads {2c, 2c+1}, kv-head c; wq/wk/wv column
shards, wo row shard. x is replicated (pre-transposed on host so the
contraction dim lands on SBUF partitions). Each core emits a partial
[4096, 2048] o-proj product; the host gather for the row-parallel o-proj
is a sum over the 8 partials.

Device dataflow (per core):
  A) QKV^T projections ([d, tok] layout) via float32r matmuls; one ACT
     copy evicts each PSUM head slab to SBUF; RMSNorm sum-of-squares via
     GPSIMD partition-allreduce (the q/k norm weights are folded into the
     RoPE tables on the host); RoPE as partition-half shuffle; the rstd
     scale is applied after RoPE (commutes -- rstd is column-uniform).
     V is transposed back to natural [tok, d] via PE transposes.
  B) Causal attention, two sub-phases per (batch, q-tile, head):
     (1) S^T tiles [128 k, 512 q] = K^T_tile.T @ Q^T + exp on ACT (no max
         subtraction -- RMSNorm bounds |scores| <= sqrt(128)) + causal
         affine_select on the diagonal band;
     (2) softmax denominator (ones-vector matmuls) and PV (V_nat as
         stationary) accumulated over k-tiles.
     Then the row-parallel o-proj partial, streamed out per 512-row tile.
"""

import math
import os
import sys

import numpy as np

for _p in ("/opt/trn_rl_repo", "/root/.axon_site/_ro/trn_rl_repo"):
    if os.path.isdir(_p) and _p not in sys.path:
        sys.path.insert(0, _p)
        break

import concourse.bacc as bacc
import concourse.tile as tile
from concourse import mybir
from concourse.bass_isa import ReduceOp
from concourse.bass_utils import run_bass_kernel_spmd
from concourse.masks import make_identity

# Problem constants (hardcoded per contract)
B, S, HID = 2, 2048, 2048
NH, NKV, D = 16, 8, 128
NCORES = 8
HQ = NH // NCORES  # q heads per core = 2
T = B * S          # 4096 tokens
EPS = 1e-5
F32 = mybir.dt.float32
F32R = mybir.dt.float32r
BF16 = mybir.dt.bfloat16
# matmul input dtype: "f32r" (near-fp32, default) or "bf16" (halves phase-A
# DMA; ~1e-3-class output error)
KDT = os.environ.get("BASS_KDT", "f32r")
MDT = BF16 if KDT == "bf16" else F32R
NP_MDT = None  # set lazily in prep_inputs (ml_dtypes import)
# transpose path (identity matmul) dtype: f32r can't be memset/ldweights'd,
# so use plain f32 there in f32r mode
TDT = BF16 if KDT == "bf16" else F32
SCALE = 1.0 / math.sqrt(D)

KT = HID // 128      # 16 contraction tiles
TT = T // 512        # 8 token tiles of 512
QT_PER_B = S // 512  # 4 q-tiles per batch


def build_nc():
    nc = bacc.Bacc("TRN2", target_bir_lowering=False, debug=False)
    xt = nc.dram_tensor("xt", [HID, T], MDT, kind="ExternalInput").ap()
    wqkv = nc.dram_tensor("wqkv", [HID, 4 * D], MDT, kind="ExternalInput").ap()
    woc = nc.dram_tensor("woc", [HQ * D, HID], MDT, kind="ExternalInput").ap()
    pmat = nc.dram_tensor("pmat", [D, D], MDT, kind="ExternalInput").ap()
    onec = nc.dram_tensor("onec", [D, 1], MDT, kind="ExternalInput").ap()
    ctq = nc.dram_tensor("ctq", [D, S], MDT, kind="ExternalInput").ap()
    stq = nc.dram_tensor("stq", [D, S], MDT, kind="ExternalInput").ap()
    ctk = nc.dram_tensor("ctk", [D, S], MDT, kind="ExternalInput").ap()
    stk = nc.dram_tensor("stk", [D, S], MDT, kind="ExternalInput").ap()
    out = nc.dram_tensor("out", [T, HID], F32, kind="ExternalOutput").ap()

    with tile.TileContext(nc) as tc:
        from contextlib import ExitStack

        with ExitStack() as root:
            const = root.enter_context(tc.tile_pool(name="const", bufs=1))
            ident = const.tile([128, 128], TDT, name="ident")
            make_identity(nc, ident)
            ones_col = const.tile([128, 1], MDT, name="ones_col")
            nc.scalar.dma_start(out=ones_col, in_=onec)
            pmat_sb = const.tile([D, D], MDT, name="pmat_sb")
            nc.scalar.dma_start(out=pmat_sb, in_=pmat)
            eps_col = const.tile([128, 1], F32, name="eps_col")
            nc.vector.memset(eps_col, EPS)

            res = root.enter_context(tc.tile_pool(name="res", bufs=1))
            wo_sb = res.tile([128, HQ, HID], MDT, name="wo_sb")
            qt_sb = res.tile([128, HQ, T], MDT, name="qt_sb")   # [d, h, tok]
            kt_sb = res.tile([128, T], MDT, name="kt_sb")       # [d, tok]
            v_sb = res.tile([128, T // 128, D], MDT, name="v_sb")  # [tok%128, tile, d]

            # ---------------- Phase A: QKV^T, norm, rope, V transpose ---------
            with ExitStack() as pa:
                wqp = pa.enter_context(tc.tile_pool(name="wqp", bufs=1))
                xp = pa.enter_context(tc.tile_pool(name="xp", bufs=17))
                tabp = pa.enter_context(tc.tile_pool(name="tabp", bufs=2))
                wp = pa.enter_context(tc.tile_pool(name="wp", bufs=2))
                psA = pa.enter_context(tc.tile_pool(name="psA", bufs=2, space="PSUM"))
                psT = pa.enter_context(tc.tile_pool(name="psT", bufs=2, space="PSUM"))
                psR = pa.enter_context(tc.tile_pool(name="psR", bufs=2, space="PSUM"))

                wqkv_sb = wqp.tile([128, KT, 4 * D], MDT, name="wqkv_sb")

                # visit token tiles as (b0, b1) pairs sharing a sequence
                # position so each RoPE table slice is fetched once
                tabs = {}
                for ti, t in enumerate((0, 4, 1, 5, 2, 6, 3, 7)):
                    xks = []
                    for k in range(KT):
                        if ti == 0:  # interleave weight loads with first x tiles
                            nc.sync.dma_start(
                                out=wqkv_sb[:, k, :], in_=wqkv[k * 128:(k + 1) * 128, :]
                            )
                        xk = xp.tile([128, 512], MDT, name="xk", tag="xk")
                        nc.sync.dma_start(
                            out=xk, in_=xt[k * 128:(k + 1) * 128, t * 512:(t + 1) * 512]
                        )
                        xks.append(xk)
                    if ti == 5:  # wo is not needed until phase B
                        nc.sync.dma_start(
                            out=wo_sb, in_=woc.rearrange("(h p) n -> p h n", p=128)
                        )
                    # two 2-bank PSUM slabs: (q0,q1) and (k,v)
                    slabs = []
                    for g in range(2):
                        ps = psA.tile([128, 2, 512], F32, name="ps_qkv", tag="ps_qkv")
                        for k in range(KT):
                            for mm in range(2):
                                m = g * 2 + mm
                                nc.tensor.matmul(
                                    ps[:, mm, :],
                                    lhsT=(wqkv_sb[:, k, m * 128:(m + 1) * 128]),
                                    rhs=(xks[k]),
                                    start=(k == 0),
                                    stop=(k == KT - 1),
                                )
                        slabs.append(ps)

                    s0 = (t % QT_PER_B) * 512  # position-in-sequence of this tile
                    if ti % 2 == 0:  # second tile of each pair reuses the slices
                        tabs = {}
                        for nm, ap in (("cq", ctq), ("sq", stq), ("ck", ctk), ("sk", stk)):
                            tl = tabp.tile([128, 512], MDT, name="tab_" + nm, tag="tab_" + nm)
                            nc.sync.dma_start(out=tl, in_=ap[:, s0:s0 + 512])
                            tabs[nm] = tl
                    for m, cosT, sinT in (
                        (0, tabs["cq"], tabs["sq"]),
                        (1, tabs["cq"], tabs["sq"]),
                        (2, tabs["ck"], tabs["sk"]),
                    ):
                        src = slabs[m // 2][:, m % 2, :]
                        qk = wp.tile([128, 512], MDT, name="qk", tag="qk")
                        nc.scalar.copy(qk, src)  # sole PSUM reader (ACT)
                        sq = wp.tile([128, 512], F32, name="sq", tag="sq")
                        nc.vector.tensor_mul(sq, qk, qk)
                        nc.gpsimd.partition_all_reduce(sq, sq, 128, ReduceOp.add)
                        rrow = wp.tile([1, 512], F32, name="rrow", tag="rrow")
                        nc.scalar.activation(
                            rrow, sq[0:1, :], mybir.ActivationFunctionType.Sqrt,
                            bias=eps_col[0:1, :], scale=1.0 / D,
                        )
                        nc.vector.reciprocal(rrow, rrow)
                        rstd = wp.tile([128, 512], F32, name="rstd", tag="rstd")
                        nc.gpsimd.partition_broadcast(rstd, rrow)
                        shf = psR.tile([128, 512], F32, name="shf", tag="shf")
                        nc.tensor.matmul(shf, lhsT=pmat_sb, rhs=qk, start=True, stop=True)
                        t0 = wp.tile([128, 512], F32, name="t0", tag="t0")
                        nc.vector.tensor_mul(t0, qk, cosT)
                        t1 = wp.tile([128, 512], F32, name="t1", tag="t1")
                        nc.vector.tensor_mul(t1, shf, sinT)
                        tr = wp.tile([128, 512], F32, name="tr", tag="tr")
                        nc.vector.tensor_add(tr, t0, t1)
                        if m < 2:
                            dst = qt_sb[:, m, t * 512:(t + 1) * 512]
                        else:
                            dst = kt_sb[:, t * 512:(t + 1) * 512]
                        nc.vector.tensor_mul(dst, tr, rstd)
                    # V: evict transposed VT then PE-transpose to natural
                    vt = wp.tile([128, 512], TDT, name="vt", tag="vt")
                    nc.scalar.copy(vt, slabs[1][:, 1, :])
                    for j in range(4):
                        pv = psT.tile([128, 128], TDT, name="pv", tag="pv")
                        nc.tensor.transpose(pv, vt[:, j * 128:(j + 1) * 128], ident)
                        nc.scalar.copy(v_sb[:, t * 4 + j, :], pv)

            # ---------------- Phase B: causal attention + o-proj --------------
            with ExitStack() as pb:
                ep = pb.enter_context(tc.tile_pool(name="ep", bufs=20))
                wp2 = pb.enter_context(tc.tile_pool(name="wp2", bufs=3))
                atp = pb.enter_context(tc.tile_pool(name="atp", bufs=8))
                op = pb.enter_context(tc.tile_pool(name="op", bufs=4))
                psS = pb.enter_context(tc.tile_pool(name="psS", bufs=3, space="PSUM"))
                psO = pb.enter_context(tc.tile_pool(name="psO", bufs=2, space="PSUM"))
                psD = pb.enter_context(tc.tile_pool(name="psD", bufs=1, space="PSUM"))
                psP = pb.enter_context(tc.tile_pool(name="psP", bufs=2, space="PSUM"))

                for b in range(B):
                    for qt in range(QT_PER_B):
                        q0 = qt * 512
                        at_tiles = {}
                        for h in range(HQ):
                            for qh in range(2):  # 256-wide q slices
                                qq0 = q0 + qh * 256
                                n_kt = (qq0 + 256) // 128  # valid k tiles
                                # sub-phase 1: scores, two k-tiles packed
                                # per PSUM bank, one exp per pair, causal mask
                                ets = [None] * n_kt
                                for kp in range(n_kt // 2):
                                    st = psS.tile([128, 2, 256], F32, name="st", tag="st")
                                    for j in range(2):
                                        kt = 2 * kp + j
                                        nc.tensor.matmul(
                                            st[:, j, :],
                                            lhsT=(kt_sb[:, b * S + kt * 128: b * S + (kt + 1) * 128]),
                                            rhs=(qt_sb[:, h, b * S + qq0: b * S + qq0 + 256]),
                                            start=True, stop=True,
                                        )
                                    etp = ep.tile([128, 2, 256], MDT, name="et", tag="et")
                                    nc.scalar.activation(
                                        etp, st, mybir.ActivationFunctionType.Exp,
                                        scale=SCALE,
                                    )
                                    for j in range(2):
                                        kt = 2 * kp + j
                                        et = etp[:, j, :]
                                        if kt * 128 + 127 > qq0:  # diagonal band
                                            nc.gpsimd.affine_select(
                                                out=et, in_=et,
                                                pattern=[[1, 256]],
                                                channel_multiplier=-1,
                                                base=-(kt * 128 - qq0),
                                                compare_op=mybir.AluOpType.is_ge,
                                                fill=0.0,
                                            )
                                        ets[kt] = et
                                # sub-phase 2: denominator + PV accumulation
                                ot = psO.tile([128, 256], F32, name="ot", tag="ot")
                                den = psD.tile([1, 256], F32, name="den", tag="den")
                                for kt in range(n_kt):
                                    nc.tensor.matmul(
                                        den, lhsT=ones_col, rhs=ets[kt],
                                        start=(kt == 0), stop=(kt == n_kt - 1),
                                    )
                                    nc.tensor.matmul(
                                        ot, lhsT=(v_sb[:, b * (S // 128) + kt, :]),
                                        rhs=(ets[kt]),
                                        start=(kt == 0), stop=(kt == n_kt - 1),
                                    )
                                rd = wp2.tile([1, 256], F32, name="rd", tag="rd")
                                nc.vector.reciprocal(rd, den)
                                rb = wp2.tile([128, 256], F32, name="rb", tag="rb")
                                nc.gpsimd.partition_broadcast(rb, rd)
                                at = atp.tile([128, 256], MDT, name="at", tag="at")
                                nc.vector.tensor_mul(at, ot, rb)
                                at_tiles[(h, qh)] = at
                        # o-proj partial for rows [b*S+q0, +512)
                        for mq in range(4):
                            qh = mq // 2
                            mq2 = mq % 2  # 128-slice within the 256 at tile
                            for nn in range(4):
                                po = psP.tile([128, 512], F32, name="po", tag="po")
                                for h in range(HQ):
                                    nc.tensor.matmul(
                                        po,
                                        lhsT=(at_tiles[(h, qh)][:, mq2 * 128:(mq2 + 1) * 128]),
                                        rhs=(wo_sb[:, h, nn * 512:(nn + 1) * 512]),
                                        start=(h == 0), stop=(h == HQ - 1),
                                    )
                                ob = op.tile([128, 512], F32, name="ob", tag="ob")
                                # batch 1: ACT has slack (phase-A tail done) and
                                # DVE is the mid-phase-B choke; batch 0: keep DVE
                                if b == 1 and (mq + nn) % 2 == 0:
                                    nc.scalar.copy(ob, po)
                                else:
                                    nc.vector.tensor_copy(ob, po)
                                nc.sync.dma_start(
                                    out=out[b * S + q0 + mq * 128: b * S + q0 + (mq + 1) * 128,
                                            nn * 512:(nn + 1) * 512],
                                    in_=ob,
                                )
    nc.compile()
    return nc


def _rot_half(w):
    return np.concatenate([w[D // 2:], w[:D // 2]])


def prep_inputs(x, cos, sin, wq, wk, wv, wo, q_norm_w, k_norm_w):
    """Host-side sharding/layout prep. Returns per-core in_maps."""
    f = np.float32
    if KDT == "bf16":
        import ml_dtypes
        mf = np.dtype(ml_dtypes.bfloat16)
    else:
        mf = np.float32
    cvt = lambda a: np.ascontiguousarray(a.astype(mf))
    x = np.asarray(x, f)
    cos = np.asarray(cos, f)
    sin = np.asarray(sin, f)
    wq, wk, wv, wo = (np.asarray(a, f) for a in (wq, wk, wv, wo))
    q_norm_w = np.asarray(q_norm_w, f)
    k_norm_w = np.asarray(k_norm_w, f)

    xt = np.ascontiguousarray(x.reshape(T, HID).T)  # [HID, T]
    ctq = np.ascontiguousarray(cos.T * q_norm_w[:, None])
    stq = np.ascontiguousarray(sin.T * _rot_half(q_norm_w)[:, None])
    ctk = np.ascontiguousarray(cos.T * k_norm_w[:, None])
    stk = np.ascontiguousarray(sin.T * _rot_half(k_norm_w)[:, None])
    # rotate-half permutation (with sign) as a matmul stationary operand:
    # out[d] = sum_j pmat[j, d] * q[j] = sign(d) * q[(d+64) % 128]
    pmat = np.zeros((D, D), f)
    for d in range(D // 2):
        pmat[d + D // 2, d] = -1.0
    for d in range(D // 2, D):
        pmat[d - D // 2, d] = 1.0
    onec = np.ones((D, 1), f)
    xt_m, ctq_m, stq_m, ctk_m, stk_m, pmat_m, onec_m = (
        cvt(a) for a in (xt, ctq, stq, ctk, stk, pmat, onec))

    in_maps = []
    for c in range(NCORES):
        wqkv_c = np.ascontiguousarray(np.concatenate([
            wq[:, c * HQ * D:(c + 1) * HQ * D],
            wk[:, c * D:(c + 1) * D],
            wv[:, c * D:(c + 1) * D],
        ], axis=1))
        woc = np.ascontiguousarray(wo[c * HQ * D:(c + 1) * HQ * D, :])
        in_maps.append({
            "xt": xt_m, "wqkv": cvt(wqkv_c), "woc": cvt(woc),
            "pmat": pmat_m, "onec": onec_m,
            "ctq": ctq_m, "stq": stq_m, "ctk": ctk_m, "stk": stk_m,
        })
    return in_maps


_NC = None


def get_nc():
    global _NC
    if _NC is None:
        _NC = build_nc()
    return _NC


def kernel(x, cos, sin, wq, wk, wv, wo, q_norm_w, k_norm_w):
    nc = get_nc()
    in_maps = prep_inputs(x, cos, sin, wq, wk, wv, wo, q_norm_w, k_norm_w)
    res = run_bass_kernel_spmd(nc, in_maps, core_ids=list(range(NCORES)))
    acc = np.zeros((T, HID), dtype=np.float64)
    for c in range(NCORES):
        acc += res.results[c]["out"]
    return acc.astype(np.float32).reshape(B, S, HID)



# revision 27
# speedup vs baseline: 1.2017x; 1.2017x over previous
"""Trainium2 Bass kernel for a GQA attention block (B=2, S=2048, H=2048,
16 q-heads / 8 kv-heads, head_dim=128, fp32), tensor-parallel over heads
across 8 NeuronCores.

Per-core shard (core c): q-heads {2c, 2c+1}, kv-head c; wq/wk/wv column
shards, wo row shard. x is replicated (pre-transposed to [HID, T] bf16 on
host). Each core emits a partial [4096, 2048] f32 o-proj product; the host
gather sums the 8 partials.

Device dataflow (per core), all matmul inputs bf16:
  A) QKV^T projections ([d, tok] layout): x streamed as [128, 1024] bf16
     chunks (two 512-token tiles per DMA), RoPE tables resident in SBUF.
     Per 512-token tile and head: one ACT copy evicts the PSUM slab;
     RMSNorm sum-of-squares on DVE (bf16) + GPSIMD partition-allreduce
     (result in ALL partitions); rstd = Abs_reciprocal_sqrt on ACT over the
     full tile (no partition broadcast, no DVE reciprocal); RoPE as
     partition-half shuffle matmul; rstd applied after RoPE (commutes).
     V^T slabs are evicted to bf16 and transposed to natural [tok, d]
     via DMA-xbar transpose (no PE transposes).
  B) Causal attention per (batch, q-tile, head, 256-q slice):
     scores S^T [128 k, 256 q] matmuls two-packed per PSUM bank; exp on
     ACT (no max subtraction -- RMSNorm bounds |scores| <= sqrt(128));
     causal affine_select on the diagonal band; softmax denominator on
     DVE (bf16 pair-adds) + one GPSIMD allreduce + full-tile reciprocal
     (PE does NOT compute the denominator); PV accumulated over k-tiles;
     o-proj partials [128, 512] are DMA'd PSUM -> DRAM directly (f32, no
     SBUF staging).
"""

import math
import os
import sys

import numpy as np

for _p in ("/opt/trn_rl_repo", "/root/.axon_site/_ro/trn_rl_repo"):
    if os.path.isdir(_p) and _p not in sys.path:
        sys.path.insert(0, _p)
        break

import concourse.bacc as bacc
import concourse.tile as tile
from concourse import mybir
from concourse.bass_isa import ReduceOp
from concourse.bass_utils import run_bass_kernel_spmd

# Problem constants (hardcoded per contract)
B, S, HID = 2, 2048, 2048
NH, NKV, D = 16, 8, 128
NCORES = 8
HQ = NH // NCORES  # q heads per core = 2
T = B * S          # 4096 tokens
EPS = 1e-5
F32 = mybir.dt.float32
BF16 = mybir.dt.bfloat16
MDT = BF16
SCALE = 1.0 / math.sqrt(D)
# rstd on ACT via Abs_reciprocal_sqrt (1/sqrt(|x|), exact for x>=0); set to
# 0 to fall back to Sqrt + DVE reciprocal if HW accuracy disappoints
USE_ARS = os.environ.get("BASS_ARS", "1") == "1"

KT = HID // 128      # 16 contraction tiles
TT = T // 512        # 8 token tiles of 512
QT_PER_B = S // 512  # 4 q-tiles per batch


def build_nc():
    nc = bacc.Bacc("TRN2", target_bir_lowering=False, debug=False)
    xt = nc.dram_tensor("xt", [HID, T], MDT, kind="ExternalInput").ap()
    wqkv = nc.dram_tensor("wqkv", [HID, 4 * D], MDT, kind="ExternalInput").ap()
    woc = nc.dram_tensor("woc", [HQ * D, HID], MDT, kind="ExternalInput").ap()
    pmat = nc.dram_tensor("pmat", [D, D], MDT, kind="ExternalInput").ap()
    # 4 RoPE tables (q-cos, q-sin, k-cos, k-sin), norm weights folded in
    tab4 = nc.dram_tensor("tab4", [D, 4, S], MDT, kind="ExternalInput").ap()
    # partials are summed across cores on the host; bf16 partials keep the
    # final error ~0.4% of partial RMS, well inside the 2e-2 budget
    out = nc.dram_tensor("out", [T, HID], MDT, kind="ExternalOutput").ap()

    with tile.TileContext(nc) as tc:
        from contextlib import ExitStack

        with ExitStack() as root:
            const = root.enter_context(tc.tile_pool(name="const", bufs=1))
            pmat_sb = const.tile([D, D], MDT, name="pmat_sb")
            eps_col = const.tile([128, 1], F32, name="eps_col")
            nc.vector.memset(eps_col, EPS)
            # causal masks for the two diagonal-band k-tiles of a 256-q
            # slice: cmask[:, j, q] = 1 iff q >= p + 128*j. Applied as a DVE
            # multiply (cheaper + off the Pool critical path vs affine_select)
            cmask = const.tile([128, 2, 256], MDT, name="cmask")
            nc.vector.memset(cmask, 1.0)
            for j in range(2):
                nc.gpsimd.affine_select(
                    out=cmask[:, j, :], in_=cmask[:, j, :],
                    pattern=[[1, 256]], channel_multiplier=-1, base=-128 * j,
                    compare_op=mybir.AluOpType.is_ge, fill=0.0,
                )

            res = root.enter_context(tc.tile_pool(name="res", bufs=1))
            wo_sb = res.tile([128, HQ, HID], MDT, name="wo_sb")
            qt_sb = res.tile([128, HQ, T], MDT, name="qt_sb")   # [d, h, tok]
            kt_sb = res.tile([128, T], MDT, name="kt_sb")       # [d, tok]
            v_sb = res.tile([128, T // 128, D], MDT, name="v_sb")  # [tok%128, tile, d]
            tab_sb = res.tile([128, 4, S], MDT, name="tab_sb")

            # ---------------- Phase A: QKV^T, norm, rope, V transpose ---------
            with ExitStack() as pa:
                wqp = pa.enter_context(tc.tile_pool(name="wqp", bufs=1))
                xp = pa.enter_context(tc.tile_pool(name="xp", bufs=33))
                xp5 = pa.enter_context(tc.tile_pool(name="xp5", bufs=17))
                wp = pa.enter_context(tc.tile_pool(name="wp", bufs=2))
                # 1-bank spacer: phase B's first score banks then land on
                # PSUM never touched by phase A (no release-sync stall)
                pad = pa.enter_context(tc.tile_pool(name="pad", bufs=1, space="PSUM"))
                pad.tile([128, 512], F32, name="pad0")
                psA = pa.enter_context(tc.tile_pool(name="psA", bufs=2, space="PSUM"))
                psR = pa.enter_context(tc.tile_pool(name="psR", bufs=2, space="PSUM"))

                wqkv_sb = wqp.tile([128, KT, 4 * D], MDT, name="wqkv_sb")

                # x streaming. The first two 512-token tiles are loaded as
                # individual [128, 512] slices so the DMA engine keeps pace
                # with the PE during warmup (RoPE table quarters slot into
                # tile 1's stream); later tiles use [128, 1024] chunks.
                xviews = {}  # t -> per-k list of (tile, base_offset)

                def load_tile_split(t):
                    # two k-slices per DMA keeps the HWDGE issue rate
                    # (625ns/DMA, globally serialized) ahead of the PE
                    lst = []
                    for kp in range(KT // 2):
                        k = 2 * kp
                        if t == 0 and kp % 2 == 0:  # wqkv in 4-k groups
                            nc.sync.dma_start(
                                out=wqkv_sb[:, k:k + 4, :],
                                in_=wqkv[k * 128:(k + 4) * 128, :].rearrange(
                                    "(a p) n -> p a n", p=128),
                            )
                        xk = xp5.tile([128, 2, 512], MDT, name="xk5", tag="xk5")
                        nc.sync.dma_start(
                            out=xk,
                            in_=xt[k * 128:(k + 2) * 128, t * 512:(t + 1) * 512].rearrange(
                                "(a p) n -> p a n", p=128),
                        )
                        lst.append((xk[:, 0, :], 0))
                        lst.append((xk[:, 1, :], 0))
                        if t == 1 and kp % 2 == 1:
                            q = kp // 2
                            nc.sync.dma_start(out=tab_sb[:, q, :], in_=tab4[:, q, :])
                    xviews[t] = lst

                def load_chunk(c):  # tiles 2c, 2c+1 (c >= 1)
                    lst = []
                    for k in range(KT):
                        xk = xp.tile([128, 1024], MDT, name="xk", tag="xk")
                        nc.sync.dma_start(
                            out=xk, in_=xt[k * 128:(k + 1) * 128, c * 1024:(c + 1) * 1024]
                        )
                        lst.append(xk)
                    xviews[2 * c] = [(xk, 0) for xk in lst]
                    xviews[2 * c + 1] = [(xk, 512) for xk in lst]

                load_tile_split(0)
                # pmat off the critical first HWDGE slots; needed at ~15us
                nc.scalar.dma_start(out=pmat_sb, in_=pmat)
                load_tile_split(1)

                for t in range(TT):
                    if t % 2 == 0 and t + 2 < TT:
                        load_chunk(t // 2 + 1)
                    if t == 2:  # wo is not needed until phase B
                        nc.sync.dma_start(
                            out=wo_sb, in_=woc.rearrange("(h p) n -> p h n", p=128)
                        )
                    # two 2-bank PSUM slabs: (q0,q1) and (k,v)
                    slabs = []
                    for g in range(2):
                        ps = psA.tile([128, 2, 512], F32, name="ps_qkv", tag="ps_qkv")
                        for k in range(KT):
                            xk, base = xviews[t][k]
                            for mm in range(2):
                                m = g * 2 + mm
                                nc.tensor.matmul(
                                    ps[:, mm, :],
                                    lhsT=(wqkv_sb[:, k, m * 128:(m + 1) * 128]),
                                    rhs=(xk[:, base:base + 512]),
                                    start=(k == 0),
                                    stop=(k == KT - 1),
                                )
                        slabs.append(ps)

                    s0 = (t % QT_PER_B) * 512  # position-in-sequence
                    for m in range(3):  # q0, q1, k
                        ti = 0 if m < 2 else 2  # cos table index (q vs k)
                        cosT = tab_sb[:, ti, s0:s0 + 512]
                        sinT = tab_sb[:, ti + 1, s0:s0 + 512]
                        src = slabs[m // 2][:, m % 2, :]
                        qk = wp.tile([128, 512], MDT, name="qk", tag="qk")
                        nc.scalar.copy(qk, src)  # PSUM eviction (ACT)
                        sq = wp.tile([128, 512], MDT, name="sq", tag="sq")
                        nc.vector.tensor_mul(sq, qk, qk)
                        nc.gpsimd.partition_all_reduce(sq, sq, 128, ReduceOp.add)
                        # rstd in every partition (allreduce output is
                        # replicated): no broadcast needed
                        rstd = wp.tile([128, 512], MDT, name="rstd", tag="rstd")
                        if USE_ARS:
                            nc.scalar.activation(
                                rstd, sq,
                                mybir.ActivationFunctionType.Abs_reciprocal_sqrt,
                                bias=eps_col, scale=1.0 / D,
                            )
                        else:
                            rr = wp.tile([128, 512], F32, name="rr", tag="rr")
                            nc.scalar.activation(
                                rr, sq, mybir.ActivationFunctionType.Sqrt,
                                bias=eps_col, scale=1.0 / D,
                            )
                            with nc.allow_low_precision(reason="rstd bf16"):
                                nc.vector.reciprocal(rstd, rr)
                        shf = psR.tile([128, 512], F32, name="shf", tag="shf")
                        nc.tensor.matmul(shf, lhsT=pmat_sb, rhs=qk, start=True, stop=True)
                        t0 = wp.tile([128, 512], MDT, name="t0", tag="t0")
                        nc.vector.tensor_mul(t0, qk, cosT)
                        t1 = wp.tile([128, 512], MDT, name="t1", tag="t1")
                        nc.vector.tensor_mul(t1, shf, sinT)  # reads PSUM
                        tr = wp.tile([128, 512], MDT, name="tr", tag="tr")
                        nc.vector.tensor_add(tr, t0, t1)
                        if m < 2:
                            dst = qt_sb[:, m, t * 512:(t + 1) * 512]
                        else:
                            dst = kt_sb[:, t * 512:(t + 1) * 512]
                        nc.vector.tensor_mul(dst, tr, rstd)
                    # V: evict transposed VT (bf16) then DMA-xbar transpose
                    # to natural [tok, d]
                    vt = wp.tile([128, 512], MDT, name="vt", tag="vt")
                    nc.scalar.copy(vt, slabs[1][:, 1, :])
                    nc.sync.dma_start_transpose(
                        v_sb[:, t * 4:(t + 1) * 4, :], vt
                    )

            # ---------------- Phase B: causal attention + o-proj --------------
            with ExitStack() as pb:
                ep = pb.enter_context(tc.tile_pool(name="ep", bufs=20))
                wp2 = pb.enter_context(tc.tile_pool(name="wp2", bufs=3))
                atp = pb.enter_context(tc.tile_pool(name="atp", bufs=8))
                op = pb.enter_context(tc.tile_pool(name="op", bufs=3))
                psS = pb.enter_context(tc.tile_pool(name="psS", bufs=3, space="PSUM"))
                psO = pb.enter_context(tc.tile_pool(name="psO", bufs=2, space="PSUM"))
                psP = pb.enter_context(tc.tile_pool(name="psP", bufs=3, space="PSUM"))

                def emit_oproj_mq(b, q0, at_tiles, mq):
                    # one 128-row block of the o-proj partial for rows
                    # [b*S+q0, +512): evictions rotate across DVE/ACT/Pool,
                    # one row-contiguous bf16 DMA per block
                    qh = mq // 2
                    mq2 = mq % 2  # 128-slice within the 256 at tile
                    ob4 = op.tile([128, 4, 512], MDT, name="ob4", tag="ob4")
                    for nn in range(4):
                        po = psP.tile([128, 512], F32, name="po", tag="po")
                        for h in range(HQ):
                            nc.tensor.matmul(
                                po,
                                lhsT=(at_tiles[(h, qh)][:, mq2 * 128:(mq2 + 1) * 128]),
                                rhs=(wo_sb[:, h, nn * 512:(nn + 1) * 512]),
                                start=(h == 0), stop=(h == HQ - 1),
                            )
                        dst = ob4[:, nn, :]
                        # GPSIMD cannot read PSUM on HW: DVE/ACT alternate
                        if (mq * 4 + nn) % 2 == 0:
                            nc.vector.tensor_copy(dst, po)
                        else:
                            nc.scalar.copy(dst, po)
                    nc.sync.dma_start(
                        out=out[b * S + q0 + mq * 128: b * S + q0 + (mq + 1) * 128, :],
                        in_=ob4,
                    )

                # o-proj runs one q-tile behind the attention slices, one
                # 128-row chunk emitted between each slice's scores and PV
                # as PE filler for the exp/mask/denominator tail. q-tiles go
                # largest-first so B starts with plenty of score work.
                pending = None
                for b in range(B):
                    for qt in range(QT_PER_B):
                        q0 = qt * 512
                        at_tiles = {}
                        slice_idx = 0
                        for h in range(HQ):
                            for qh in range(2):  # 256-wide q slices
                                qq0 = q0 + qh * 256
                                n_kt = (qq0 + 256) // 128  # valid k tiles
                                # sub-phase 1: scores, two k-tiles packed
                                # per PSUM bank, one exp per pair, causal mask
                                ets = [None] * n_kt
                                etps = []
                                for kp in range(n_kt // 2):
                                    st = psS.tile([128, 2, 256], F32, name="st", tag="st")
                                    for j in range(2):
                                        kt = 2 * kp + j
                                        nc.tensor.matmul(
                                            st[:, j, :],
                                            lhsT=(kt_sb[:, b * S + kt * 128: b * S + (kt + 1) * 128]),
                                            rhs=(qt_sb[:, h, b * S + qq0: b * S + qq0 + 256]),
                                            start=True, stop=True,
                                        )
                                    etp = ep.tile([128, 2, 256], MDT, name="et", tag="et")
                                    nc.scalar.activation(
                                        etp, st, mybir.ActivationFunctionType.Exp,
                                        scale=SCALE,
                                    )
                                    for j in range(2):
                                        kt = 2 * kp + j
                                        et = etp[:, j, :]
                                        if kt * 128 + 127 > qq0:  # diagonal band
                                            msk = cmask[:, (kt * 128 - qq0) // 128, :]
                                            nc.vector.tensor_mul(et, et, msk)
                                        ets[kt] = et
                                    etps.append(etp)
                                # softmax denominator on DVE + Pool (no PE):
                                # pair-sum each etp, accumulate, allreduce
                                acc = wp2.tile([128, 256], MDT, name="acc", tag="acc")
                                tmp = wp2.tile([128, 256], MDT, name="tmp", tag="tmp")
                                for i, etp in enumerate(etps):
                                    dst = acc if i == 0 else tmp
                                    nc.vector.tensor_add(dst, etp[:, 0, :], etp[:, 1, :])
                                    if i > 0:
                                        nc.vector.tensor_add(acc, acc, tmp)
                                nc.gpsimd.partition_all_reduce(acc, acc, 128, ReduceOp.add)
                                rb = wp2.tile([128, 256], MDT, name="rb", tag="rb")
                                with nc.allow_low_precision(reason="1/den bf16"):
                                    nc.vector.reciprocal(rb, acc)
                                # sub-phase 2: PV accumulation over k-tiles
                                ot = psO.tile([128, 256], F32, name="ot", tag="ot")
                                for kt in range(n_kt):
                                    nc.tensor.matmul(
                                        ot, lhsT=(v_sb[:, b * (S // 128) + kt, :]),
                                        rhs=(ets[kt]),
                                        start=(kt == 0), stop=(kt == n_kt - 1),
                                    )
                                at = atp.tile([128, 256], MDT, name="at", tag="at")
                                nc.vector.tensor_mul(at, ot, rb)
                                at_tiles[(h, qh)] = at
                                slice_idx += 1
                        if pending is not None:
                            for mq in range(4):
                                emit_oproj_mq(*pending, mq)
                        pending = (b, q0, at_tiles)
                for mq in range(4):
                    emit_oproj_mq(*pending, mq)
    nc.compile()
    return nc


def _rot_half(w):
    return np.concatenate([w[D // 2:], w[:D // 2]])


def prep_inputs(x, cos, sin, wq, wk, wv, wo, q_norm_w, k_norm_w):
    """Host-side sharding/layout prep. Returns per-core in_maps."""
    import ml_dtypes
    f = np.float32
    mf = np.dtype(ml_dtypes.bfloat16)
    cvt = lambda a: np.ascontiguousarray(a.astype(mf))
    x = np.asarray(x, f)
    cos = np.asarray(cos, f)
    sin = np.asarray(sin, f)
    wq, wk, wv, wo = (np.asarray(a, f) for a in (wq, wk, wv, wo))
    q_norm_w = np.asarray(q_norm_w, f)
    k_norm_w = np.asarray(k_norm_w, f)

    xt = np.ascontiguousarray(x.reshape(T, HID).T)  # [HID, T]
    ctq = cos.T * q_norm_w[:, None]
    stq = sin.T * _rot_half(q_norm_w)[:, None]
    ctk = cos.T * k_norm_w[:, None]
    stk = sin.T * _rot_half(k_norm_w)[:, None]
    tab4 = np.stack([ctq, stq, ctk, stk], axis=1)  # [D, 4, S]
    # rotate-half permutation (with sign) as a matmul stationary operand:
    # out[d] = sum_j pmat[j, d] * q[j] = sign(d) * q[(d+64) % 128]
    pmat = np.zeros((D, D), f)
    for d in range(D // 2):
        pmat[d + D // 2, d] = -1.0
    for d in range(D // 2, D):
        pmat[d - D // 2, d] = 1.0
    xt_m, tab4_m, pmat_m = cvt(xt), cvt(tab4), cvt(pmat)

    in_maps = []
    for c in range(NCORES):
        wqkv_c = np.ascontiguousarray(np.concatenate([
            wq[:, c * HQ * D:(c + 1) * HQ * D],
            wk[:, c * D:(c + 1) * D],
            wv[:, c * D:(c + 1) * D],
        ], axis=1))
        woc = np.ascontiguousarray(wo[c * HQ * D:(c + 1) * HQ * D, :])
        in_maps.append({
            "xt": xt_m, "wqkv": cvt(wqkv_c), "woc": cvt(woc),
            "pmat": pmat_m, "tab4": tab4_m,
        })
    return in_maps


_NC = None


def get_nc():
    global _NC
    if _NC is None:
        _NC = build_nc()
    return _NC


def kernel(x, cos, sin, wq, wk, wv, wo, q_norm_w, k_norm_w):
    nc = get_nc()
    in_maps = prep_inputs(x, cos, sin, wq, wk, wv, wo, q_norm_w, k_norm_w)
    res = run_bass_kernel_spmd(nc, in_maps, core_ids=list(range(NCORES)))
    acc = np.zeros((T, HID), dtype=np.float64)
    for c in range(NCORES):
        acc += res.results[c]["out"]
    return acc.astype(np.float32).reshape(B, S, HID)


# revision 58
# speedup vs baseline: 1.2168x; 1.0126x over previous
"""Trainium2 Bass kernel for a GQA attention block (B=2, S=2048, H=2048,
16 q-heads / 8 kv-heads, head_dim=128, fp32), tensor-parallel over heads
across 8 NeuronCores.

Per-core shard (core c): q-heads {2c, 2c+1}, kv-head c; wq/wk/wv column
shards, wo row shard. x is replicated (pre-transposed to [HID, T] bf16 on
host). Each core emits a partial [4096, 2048] f32 o-proj product; the host
gather sums the 8 partials.

Device dataflow (per core), all matmul inputs bf16:
  A) QKV^T projections ([d, tok] layout): x streamed as [128, 1024] bf16
     chunks (two 512-token tiles per DMA), RoPE tables resident in SBUF.
     Per 512-token tile and head: one ACT copy evicts the PSUM slab;
     RMSNorm sum-of-squares on DVE (bf16) + GPSIMD partition-allreduce
     (result in ALL partitions); rstd = Abs_reciprocal_sqrt on ACT over the
     full tile (no partition broadcast, no DVE reciprocal); RoPE as
     partition-half shuffle matmul; rstd applied after RoPE (commutes).
     V^T slabs are evicted to bf16 and transposed to natural [tok, d]
     via DMA-xbar transpose (no PE transposes).
  B) Causal attention per (batch, q-tile, head, 256-q slice):
     scores S^T [128 k, 256 q] matmuls two-packed per PSUM bank; exp on
     ACT (no max subtraction -- RMSNorm bounds |scores| <= sqrt(128));
     causal affine_select on the diagonal band; softmax denominator on
     DVE (bf16 pair-adds) + one GPSIMD allreduce + full-tile reciprocal
     (PE does NOT compute the denominator); PV accumulated over k-tiles;
     o-proj partials [128, 512] are DMA'd PSUM -> DRAM directly (f32, no
     SBUF staging).
"""

import math
import os
import sys

import numpy as np

for _p in ("/opt/trn_rl_repo", "/root/.axon_site/_ro/trn_rl_repo"):
    if os.path.isdir(_p) and _p not in sys.path:
        sys.path.insert(0, _p)
        break

import concourse.bacc as bacc
import concourse.tile as tile
from concourse import mybir
from concourse.bass_isa import ReduceOp
from concourse.bass_utils import run_bass_kernel_spmd

# Problem constants (hardcoded per contract)
B, S, HID = 2, 2048, 2048
NH, NKV, D = 16, 8, 128
NCORES = 8
HQ = NH // NCORES  # q heads per core = 2
T = B * S          # 4096 tokens
EPS = 1e-5
F32 = mybir.dt.float32
BF16 = mybir.dt.bfloat16
MDT = BF16
SCALE = 1.0 / math.sqrt(D)
# rstd on ACT via Abs_reciprocal_sqrt (1/sqrt(|x|), exact for x>=0); set to
# 0 to fall back to Sqrt + DVE reciprocal if HW accuracy disappoints
USE_ARS = os.environ.get("BASS_ARS", "1") == "1"

KT = HID // 128      # 16 contraction tiles
TT = T // 512        # 8 token tiles of 512
QT_PER_B = S // 512  # 4 q-tiles per batch


def build_nc():
    nc = bacc.Bacc("TRN2", target_bir_lowering=False, debug=False)
    xt = nc.dram_tensor("xt", [HID, T], MDT, kind="ExternalInput").ap()
    wqkv = nc.dram_tensor("wqkv", [HID, 4 * D], MDT, kind="ExternalInput").ap()
    woc = nc.dram_tensor("woc", [HQ * D, HID], MDT, kind="ExternalInput").ap()
    pmat = nc.dram_tensor("pmat", [D, D], MDT, kind="ExternalInput").ap()
    # 4 RoPE tables (q-cos, q-sin, k-cos, k-sin), norm weights folded in
    tab4 = nc.dram_tensor("tab4", [D, 4, S], MDT, kind="ExternalInput").ap()
    # partials are summed across cores on the host; bf16 partials keep the
    # final error ~0.4% of partial RMS, well inside the 2e-2 budget
    out = nc.dram_tensor("out", [T, HID], MDT, kind="ExternalOutput").ap()

    with tile.TileContext(nc) as tc:
        from contextlib import ExitStack

        with ExitStack() as root:
            const = root.enter_context(tc.tile_pool(name="const", bufs=1))
            pmat_sb = const.tile([D, D], MDT, name="pmat_sb")
            eps_col = const.tile([128, 1], F32, name="eps_col")
            nc.vector.memset(eps_col, EPS)
            # causal masks for the two diagonal-band k-tiles of a 256-q
            # slice: cmask[:, j, q] = 1 iff q >= p + 128*j. Applied as a DVE
            # multiply (cheaper + off the Pool critical path vs affine_select)
            cmask = const.tile([128, 2, 256], MDT, name="cmask")
            nc.vector.memset(cmask, 1.0)
            for j in range(2):
                nc.gpsimd.affine_select(
                    out=cmask[:, j, :], in_=cmask[:, j, :],
                    pattern=[[1, 256]], channel_multiplier=-1, base=-128 * j,
                    compare_op=mybir.AluOpType.is_ge, fill=0.0,
                )

            res = root.enter_context(tc.tile_pool(name="res", bufs=1))
            wo_sb = res.tile([128, HQ, HID], MDT, name="wo_sb")
            qt_sb = res.tile([128, HQ, T], MDT, name="qt_sb")   # [d, h, tok]
            kt_sb = res.tile([128, T], MDT, name="kt_sb")       # [d, tok]
            v_sb = res.tile([128, T // 128, D], MDT, name="v_sb")  # [tok%128, tile, d]
            tab_sb = res.tile([128, 4, S], MDT, name="tab_sb")

            # attention score-stage pools live at root (PSUM banks 0-2) so
            # (b0,qt0)'s scores can be emitted inside phase A's tail
            ep = root.enter_context(tc.tile_pool(name="ep", bufs=20))
            wp2 = root.enter_context(tc.tile_pool(name="wp2", bufs=5))
            psS = root.enter_context(tc.tile_pool(name="psS", bufs=3, space="PSUM"))
            psO = root.enter_context(tc.tile_pool(name="psO", bufs=1, space="PSUM"))
            # one persistent PV bank, halves ping-ponged by slice parity
            psO_t = psO.tile([128, 2, 256], F32, name="ot2")
            pv_count = [0]

            def slice_scores(b, q0, h, qh):
                # scores, two k-tiles packed per PSUM bank, one exp per
                # pair, causal mask, then the softmax denominator off the
                # PE: pair-sums + chain on DVE, partition allreduce on Pool
                qq0 = q0 + qh * 256
                n_kt = (qq0 + 256) // 128  # valid k tiles
                ets = [None] * n_kt
                etps = []
                for kp in range(n_kt // 2):
                    st = psS.tile([128, 2, 256], F32, name="st", tag="st")
                    for j in range(2):
                        kt = 2 * kp + j
                        nc.tensor.matmul(
                            st[:, j, :],
                            lhsT=(kt_sb[:, b * S + kt * 128: b * S + (kt + 1) * 128]),
                            rhs=(qt_sb[:, h, b * S + qq0: b * S + qq0 + 256]),
                            start=True, stop=True,
                        )
                    etp = ep.tile([128, 2, 256], MDT, name="et", tag="et")
                    nc.scalar.activation(
                        etp, st, mybir.ActivationFunctionType.Exp,
                        scale=SCALE,
                    )
                    for j in range(2):
                        kt = 2 * kp + j
                        et = etp[:, j, :]
                        if kt * 128 + 127 > qq0:  # diagonal band
                            msk = cmask[:, (kt * 128 - qq0) // 128, :]
                            nc.vector.tensor_mul(et, et, msk)
                        ets[kt] = et
                    etps.append(etp)
                acc = wp2.tile([128, 256], MDT, name="acc", tag="acc")
                tmp = wp2.tile([128, 256], MDT, name="tmp", tag="tmp")
                for i, etp in enumerate(etps):
                    dst = acc if i == 0 else tmp
                    nc.vector.tensor_add(dst, etp[:, 0, :], etp[:, 1, :])
                    if i > 0:
                        nc.vector.tensor_add(acc, acc, tmp)
                nc.gpsimd.partition_all_reduce(acc, acc, 128, ReduceOp.add)
                rb = wp2.tile([128, 256], MDT, name="rb", tag="rb")
                with nc.allow_low_precision(reason="1/den bf16"):
                    nc.vector.reciprocal(rb, acc)
                return (b, h, qh, n_kt, ets, rb)

            b_slices = [(h, qh) for h in range(HQ) for qh in range(2)]
            prestates = []

            # ---------------- Phase A: QKV^T, norm, rope, V transpose ---------
            with ExitStack() as pa:
                wqp = pa.enter_context(tc.tile_pool(name="wqp", bufs=1))
                xp = pa.enter_context(tc.tile_pool(name="xp", bufs=25))
                xp5 = pa.enter_context(tc.tile_pool(name="xp5", bufs=17))
                wp = pa.enter_context(tc.tile_pool(name="wp", bufs=2))
                psR = pa.enter_context(tc.tile_pool(name="psR", bufs=1, space="PSUM"))
                psA = pa.enter_context(tc.tile_pool(name="psA", bufs=3, space="PSUM"))

                wqkv_sb = wqp.tile([128, KT, 4 * D], MDT, name="wqkv_sb")

                # x streaming. The first two 512-token tiles are loaded as
                # individual [128, 512] slices so the DMA engine keeps pace
                # with the PE during warmup (RoPE table quarters slot into
                # tile 1's stream); later tiles use [128, 1024] chunks.
                xviews = {}  # t -> per-k list of (tile, base_offset)

                def load_wqkv(k0, nk):
                    nc.sync.dma_start(
                        out=wqkv_sb[:, k0:k0 + nk, :],
                        in_=wqkv[k0 * 128:(k0 + nk) * 128, :].rearrange(
                            "(a p) n -> p a n", p=128),
                    )

                def load_tile_split(t):
                    # two k-slices per DMA keeps the HWDGE issue rate
                    # (625ns/DMA, globally serialized) ahead of the PE;
                    # wqkv groups start small so the first matmul fires early
                    lst = []
                    for kp in range(KT // 2):
                        k = 2 * kp
                        if t == 0:
                            if kp == 0:
                                load_wqkv(0, 2)
                            elif kp == 1:
                                load_wqkv(2, 2)
                            elif kp % 2 == 0:
                                load_wqkv(2 * kp, 4)
                        xk = xp5.tile([128, 2, 512], MDT, name="xk5", tag="xk5")
                        nc.sync.dma_start(
                            out=xk,
                            in_=xt[k * 128:(k + 2) * 128, t * 512:(t + 1) * 512].rearrange(
                                "(a p) n -> p a n", p=128),
                        )
                        lst.append((xk[:, 0, :], 0))
                        lst.append((xk[:, 1, :], 0))
                        if t == 1 and kp % 2 == 1:
                            q = kp // 2
                            nc.sync.dma_start(out=tab_sb[:, q, :], in_=tab4[:, q, :])
                    xviews[t] = lst

                def load_chunk(c):  # tiles 2c, 2c+1 (c >= 1)
                    lst = []
                    for k in range(KT):
                        xk = xp.tile([128, 1024], MDT, name="xk", tag="xk")
                        nc.sync.dma_start(
                            out=xk, in_=xt[k * 128:(k + 1) * 128, c * 1024:(c + 1) * 1024]
                        )
                        lst.append(xk)
                    xviews[2 * c] = [(xk, 0) for xk in lst]
                    xviews[2 * c + 1] = [(xk, 512) for xk in lst]

                load_tile_split(0)
                # pmat off the critical first HWDGE slots; needed at ~15us
                nc.scalar.dma_start(out=pmat_sb, in_=pmat)
                load_tile_split(1)

                for t in range(TT):
                    if t % 2 == 0 and t + 2 < TT:
                        load_chunk(t // 2 + 1)
                    if t == 2:  # wo is not needed until phase B
                        nc.sync.dma_start(
                            out=wo_sb, in_=woc.rearrange("(h p) n -> p h n", p=128)
                        )
                    # four single-bank PSUM accumulators (q0, q1, k, v) on a
                    # 3-buf rotation
                    slabs = []
                    for m in range(4):
                        ps = psA.tile([128, 512], F32, name="ps_qkv", tag="ps_qkv")
                        for k in range(KT):
                            xk, base = xviews[t][k]
                            nc.tensor.matmul(
                                ps,
                                lhsT=(wqkv_sb[:, k, m * 128:(m + 1) * 128]),
                                rhs=(xk[:, base:base + 512]),
                                start=(k == 0),
                                stop=(k == KT - 1),
                            )
                        slabs.append(ps)

                    s0 = (t % QT_PER_B) * 512  # position-in-sequence
                    for m in range(3):  # q0, q1, k
                        ti = 0 if m < 2 else 2  # cos table index (q vs k)
                        cosT = tab_sb[:, ti, s0:s0 + 512]
                        sinT = tab_sb[:, ti + 1, s0:s0 + 512]
                        src = slabs[m]
                        qk = wp.tile([128, 512], MDT, name="qk", tag="qk")
                        nc.scalar.copy(qk, src)  # PSUM eviction (ACT)
                        sq = wp.tile([128, 512], MDT, name="sq", tag="sq")
                        nc.vector.tensor_mul(sq, qk, qk)
                        nc.gpsimd.partition_all_reduce(sq, sq, 128, ReduceOp.add)
                        # rstd in every partition (allreduce output is
                        # replicated): no broadcast needed
                        rstd = wp.tile([128, 512], MDT, name="rstd", tag="rstd")
                        if USE_ARS:
                            nc.scalar.activation(
                                rstd, sq,
                                mybir.ActivationFunctionType.Abs_reciprocal_sqrt,
                                bias=eps_col, scale=1.0 / D,
                            )
                        else:
                            rr = wp.tile([128, 512], F32, name="rr", tag="rr")
                            nc.scalar.activation(
                                rr, sq, mybir.ActivationFunctionType.Sqrt,
                                bias=eps_col, scale=1.0 / D,
                            )
                            with nc.allow_low_precision(reason="rstd bf16"):
                                nc.vector.reciprocal(rstd, rr)
                        shf = psR.tile([128, 512], F32, name="shf", tag="shf")
                        nc.tensor.matmul(shf, lhsT=pmat_sb, rhs=qk, start=True, stop=True)
                        t0 = wp.tile([128, 512], MDT, name="t0", tag="t0")
                        nc.vector.tensor_mul(t0, qk, cosT)
                        t1 = wp.tile([128, 512], MDT, name="t1", tag="t1")
                        nc.vector.tensor_mul(t1, shf, sinT)  # reads PSUM
                        tr = wp.tile([128, 512], MDT, name="tr", tag="tr")
                        nc.vector.tensor_add(tr, t0, t1)
                        if m < 2:
                            dst = qt_sb[:, m, t * 512:(t + 1) * 512]
                        else:
                            dst = kt_sb[:, t * 512:(t + 1) * 512]
                        nc.vector.tensor_mul(dst, tr, rstd)
                    # V: evict transposed VT (bf16) then DMA-xbar transpose
                    # to natural [tok, d]
                    vt = wp.tile([128, 512], MDT, name="vt", tag="vt")
                    nc.scalar.copy(vt, slabs[3])
                    nc.sync.dma_start_transpose(
                        v_sb[:, t * 4:(t + 1) * 4, :], vt
                    )
                    if t == 5:
                        # (b0,qt0) attention score stage rides phase A's
                        # tail (its K/Q inputs completed with tile 3)
                        for h, qh in b_slices:
                            prestates.append(slice_scores(0, 0, h, qh))

            # ---------------- Phase B: causal attention + o-proj --------------
            with ExitStack() as pb:
                atp = pb.enter_context(tc.tile_pool(name="atp", bufs=8))
                op = pb.enter_context(tc.tile_pool(name="op", bufs=3))
                psP = pb.enter_context(tc.tile_pool(name="psP", bufs=2, space="PSUM"))

                def emit_oproj_mq(b, q0, at_tiles, mq):
                    # one 128-row block of the o-proj partial for rows
                    # [b*S+q0, +512): two-bank po pairs so each eviction
                    # moves [128, 1024] (eviction is the o-proj bottleneck
                    # at [128, 512] granularity); one bf16 DMA per block
                    qh = mq // 2
                    mq2 = mq % 2  # 128-slice within the 256 at tile
                    ob4 = op.tile([128, 4, 512], MDT, name="ob4", tag="ob4")
                    for np_ in range(2):
                        po = psP.tile([128, 2, 512], F32, name="po", tag="po")
                        for j in range(2):
                            nn = 2 * np_ + j
                            for h in range(HQ):
                                nc.tensor.matmul(
                                    po[:, j, :],
                                    lhsT=(at_tiles[(h, qh)][:, mq2 * 128:(mq2 + 1) * 128]),
                                    rhs=(wo_sb[:, h, nn * 512:(nn + 1) * 512]),
                                    start=(h == 0), stop=(h == HQ - 1),
                                )
                        dst = ob4[:, 2 * np_:2 * np_ + 2, :]
                        # GPSIMD cannot read PSUM on HW: DVE/ACT alternate
                        if (mq * 2 + np_) % 2 == 0:
                            nc.vector.tensor_copy(dst, po)
                        else:
                            nc.scalar.copy(dst, po)
                    nc.sync.dma_start(
                        out=out[b * S + q0 + mq * 128: b * S + q0 + (mq + 1) * 128, :],
                        in_=ob4,
                    )

                # o-proj runs one q-tile behind the attention slices, one
                # 128-row chunk emitted between each slice's scores and PV
                # as PE filler for the exp/mask/denominator tail. q-tiles go
                # largest-first so B starts with plenty of score work.
                def slice_pv(state, at_tiles):
                    # PV accumulation over k-tiles, then normalize
                    b, h, qh, n_kt, ets, rb = state
                    ot = psO_t[:, pv_count[0] % 2, :]
                    pv_count[0] += 1
                    for kt in range(n_kt):
                        nc.tensor.matmul(
                            ot, lhsT=(v_sb[:, b * (S // 128) + kt, :]),
                            rhs=(ets[kt]),
                            start=(kt == 0), stop=(kt == n_kt - 1),
                        )
                    at = atp.tile([128, 256], MDT, name="at", tag="at")
                    nc.vector.tensor_mul(at, ot, rb)
                    at_tiles[(h, qh)] = at

                pending = None
                for b in range(B):
                    for qt in range(QT_PER_B):
                        q0 = qt * 512
                        at_tiles = {}
                        if b == 0 and qt == 0:
                            # score stage already emitted in phase A's tail
                            for st in prestates:
                                slice_pv(st, at_tiles)
                        else:
                            for h, qh in b_slices:
                                st = slice_scores(b, q0, h, qh)
                                slice_pv(st, at_tiles)
                        if pending is not None:
                            for mq in range(4):
                                emit_oproj_mq(*pending, mq)
                        pending = (b, q0, at_tiles)
                for mq in range(4):
                    emit_oproj_mq(*pending, mq)
    nc.compile()
    return nc


def _rot_half(w):
    return np.concatenate([w[D // 2:], w[:D // 2]])


def prep_inputs(x, cos, sin, wq, wk, wv, wo, q_norm_w, k_norm_w):
    """Host-side sharding/layout prep. Returns per-core in_maps."""
    import ml_dtypes
    f = np.float32
    mf = np.dtype(ml_dtypes.bfloat16)
    cvt = lambda a: np.ascontiguousarray(a.astype(mf))
    x = np.asarray(x, f)
    cos = np.asarray(cos, f)
    sin = np.asarray(sin, f)
    wq, wk, wv, wo = (np.asarray(a, f) for a in (wq, wk, wv, wo))
    q_norm_w = np.asarray(q_norm_w, f)
    k_norm_w = np.asarray(k_norm_w, f)

    xt = np.ascontiguousarray(x.reshape(T, HID).T)  # [HID, T]
    ctq = cos.T * q_norm_w[:, None]
    stq = sin.T * _rot_half(q_norm_w)[:, None]
    ctk = cos.T * k_norm_w[:, None]
    stk = sin.T * _rot_half(k_norm_w)[:, None]
    tab4 = np.stack([ctq, stq, ctk, stk], axis=1)  # [D, 4, S]
    # rotate-half permutation (with sign) as a matmul stationary operand:
    # out[d] = sum_j pmat[j, d] * q[j] = sign(d) * q[(d+64) % 128]
    pmat = np.zeros((D, D), f)
    for d in range(D // 2):
        pmat[d + D // 2, d] = -1.0
    for d in range(D // 2, D):
        pmat[d - D // 2, d] = 1.0
    xt_m, tab4_m, pmat_m = cvt(xt), cvt(tab4), cvt(pmat)

    in_maps = []
    for c in range(NCORES):
        wqkv_c = np.ascontiguousarray(np.concatenate([
            wq[:, c * HQ * D:(c + 1) * HQ * D],
            wk[:, c * D:(c + 1) * D],
            wv[:, c * D:(c + 1) * D],
        ], axis=1))
        woc = np.ascontiguousarray(wo[c * HQ * D:(c + 1) * HQ * D, :])
        in_maps.append({
            "xt": xt_m, "wqkv": cvt(wqkv_c), "woc": cvt(woc),
            "pmat": pmat_m, "tab4": tab4_m,
        })
    return in_maps


_NC = None


def get_nc():
    global _NC
    if _NC is None:
        _NC = build_nc()
    return _NC


def kernel(x, cos, sin, wq, wk, wv, wo, q_norm_w, k_norm_w):
    nc = get_nc()
    in_maps = prep_inputs(x, cos, sin, wq, wk, wv, wo, q_norm_w, k_norm_w)
    res = run_bass_kernel_spmd(nc, in_maps, core_ids=list(range(NCORES)))
    acc = np.zeros((T, HID), dtype=np.float64)
    for c in range(NCORES):
        acc += res.results[c]["out"]
    return acc.astype(np.float32).reshape(B, S, HID)
